# revision 26
# baseline (speedup 1.0000x reference)
"""Trainium2 Bass kernel for 2-layer GCN (nn_GCN_39848706573686).

Node-sharded across 8 NeuronCores (12500 nodes/core + pad). Three SPMD
launches (host does integer routing between them; all FP math on device):
  L1: g1 = dis * (x @ W1), node-on-partition layout      (TensorE + ACT + DVE)
  L2: conv1 padded-ELL segment reduce via bf16 tree-adds
      + bias/relu/W2 epilogue                            (DVE)
  L3: conv2 padded-ELL segment reduce + bias             (DVE)

ELL slot arrays are slot-major per equal-cap segment: layout
[128 part, cap, nodes*d] so the segment reduce is a log2(cap) chain of
full-slab in-place tensor_tensor adds (DVE 2x bf16 perf mode; GPSIMD
takes the low-cap segments to unload the DVE). Segments stream and
reduce in a pipelined fashion.
"""
import os
import sys
import types
import numpy as np

# --- environment bootstrap (self-contained copy of bassboot logic) -----
for _p in ("/opt/trn_rl_repo", "/root/patched"):
    if _p not in sys.path and os.path.isdir(_p):
        sys.path.insert(0, _p)

from concourse import bass, bacc, mybir, tile  # noqa: E402
from concourse import bass_utils  # noqa: E402


def _install_ntff_hook():
    if "antenv.axon_hooks" not in sys.modules:
        mod = types.ModuleType("antenv.axon_hooks")
        _h = {}
        mod.set_axon_ntff_profile_hook = lambda h: _h.__setitem__("h", h)
        mod.get_axon_ntff_profile_hook = lambda: _h.get("h")
        sys.modules["antenv.axon_hooks"] = mod
        try:
            import antenv
            antenv.axon_hooks = mod
        except ImportError:
            pass
    mod = sys.modules["antenv.axon_hooks"]
    if mod.get_axon_ntff_profile_hook() is None:
        try:
            from trn_agent_boot.trn_boot import _ntff_profile_via_ctypes
            hook = _ntff_profile_via_ctypes("/opt/axon/libaxon_pjrt.so")
            if hook is not None:
                mod.set_axon_ntff_profile_hook(hook)
        except Exception:
            pass
    bass_utils.upload_artifacts = lambda tmpdir: str(tmpdir)


_install_ntff_hook()

# --- problem constants -------------------------------------------------
N, E, F, H = 100000, 3200000, 128, 16
NC = 8
SH = 12500                  # real nodes per core
SHP = 12544                 # padded rows per core (= 98 * 128)
NB = 98                     # node blocks of 128 per core
CAP_R = 8                   # cap rounding
SEG_MAX16 = 32768           # max slot columns per segment (d=16 plan)
SUB_COLS16 = 13312          # max slot columns per DMA sub-chunk
POOL_FRAC = float(os.environ.get("GCN_POOL", "0.0"))

FT = mybir.dt.float32
BF = mybir.dt.bfloat16
F8 = mybir.dt.float8e4
L1FP8 = os.environ.get("GCN_L1FP8", "1") == "1"
XDT = F8 if L1FP8 else BF

_cached = {}

# Track total device time across launches for test harness
last_exec_ns = {}


# ---------------------------------------------------------------------
# plan: equal-cap segments (optionally merged/split)
# ---------------------------------------------------------------------
def _runs(caps):
    runs = []
    b = 0
    while b < NB:
        b2 = b
        while b2 < NB and caps[b2] == caps[b]:
            b2 += 1
        runs.append((caps[b], b, b2 - b))
        b = b2
    return runs


def _plan_segs(caps, d, seg_max_cols, merge_min_nb=0):
    """Segment list [(cap, b0, nb, off)], consecutive in DRAM columns.

    layout inside a segment: [cap, nb*d] slot-major.
    colbase[b] + w*stride[b] + ch addresses edge slot w channel ch of
    block b.  merge_min_nb > 0 merges adjacent runs (raising cap) until a
    segment has at least that many blocks (d=1 coarse plan).
    """
    runs = _runs(caps)
    if merge_min_nb:
        merged = []
        cur = None
        for (cap, b0, nb) in runs:
            if cur is None:
                cur = [cap, b0, nb]
            else:
                cur[0] = max(cur[0], cap)
                cur[2] += nb
            if cur[2] >= merge_min_nb:
                merged.append(tuple(cur))
                cur = None
        if cur is not None:
            merged.append(tuple(cur))
        # enforce even nb (shift one block into the following segment)
        runs = []
        carry = 0
        out = []
        for i, (cap, b0, nb) in enumerate(merged):
            b0 -= carry
            nb += carry
            carry = 0
            if nb % 2 == 1 and i < len(merged) - 1:
                nb -= 1
                carry = 1
            out.append((cap, b0, nb))
        runs = [r for r in out if r[2] > 0]

    segs = []
    colbase = np.zeros(NB, np.int64)
    stride = np.zeros(NB, np.int64)
    off = 0
    for (cap, b0, nb) in runs:
        while nb > 0:
            take = min(nb, max(2, seg_max_cols // (d * cap)))
            if take % 2 == 1 and take < nb:
                take -= 1
            segs.append((cap, b0, take, off))
            for j in range(take):
                colbase[b0 + j] = off + j * d
                stride[b0 + j] = take * d
            off += take * d * cap
            b0 += take
            nb -= take
    return int(off), segs, colbase, stride


# ---------------------------------------------------------------------
# device builders
# ---------------------------------------------------------------------
def _build_l1():
    """g1 = disrep * (x @ W1) in [128 nodes, NB*16] layout."""
    PIECES = [49, 49] if L1FP8 else [33, 33, 32]    # blocks per piece
    nc = bacc.Bacc("TRN2", target_bir_lowering=False, debug=False,
                   num_devices=NC)
    xT = nc.dram_tensor("xT", [128, SHP], XDT, kind="ExternalInput").ap()
    w1 = nc.dram_tensor("w1", [128, H], XDT, kind="ExternalInput").ap()
    dis1 = nc.dram_tensor("dis1", [128, NB], BF, kind="ExternalInput").ap()
    g1 = nc.dram_tensor("g1", [128, NB * H], BF, kind="ExternalOutput").ap()
    with tile.TileContext(nc) as tc:
        with tc.tile_pool(name="sb", bufs=1) as sb, \
             tc.tile_pool(name="cst", bufs=1) as cst, \
             tc.tile_pool(name="ps", bufs=1, space="PSUM") as ps:
            w1_t = cst.tile([128, H], XDT)
            nc.scalar.dma_start(out=w1_t[:], in_=w1[:])
            dis1_t = cst.tile([128, NB], BF)
            nc.scalar.dma_start(out=dis1_t[:], in_=dis1[:])
            disrep_t = cst.tile([128, NB * H], BF)
            # replicate dis across the 16 channels on the idle GPSIMD
            nc.gpsimd.tensor_copy(
                out=disrep_t[:].rearrange("p (b c) -> p b c", b=NB, c=H),
                in_=dis1_t[:].unsqueeze(2).to_broadcast([128, NB, H]))
            g_sb = cst.tile([128, NB * H], BF)
            pieces = []
            off = 0
            pmax = max(PIECES)
            for pidx, nb_p in enumerate(PIECES):
                xt_p = sb.tile([128, pmax * 128], XDT, name=f"xtp{pidx}")
                eng = nc.sync if pidx % 2 == 0 else nc.scalar
                eng.dma_start(out=xt_p[:, :nb_p * 128],
                              in_=xT[:, off * 128:(off + nb_p) * 128])
                pieces.append((xt_p, off))
                off += nb_p
            psts = [ps.tile([128, 512], FT, space="PSUM", name=f"pst{i}")
                    for i in range(4)]
            pc = 0
            for t in range(NB):
                while t >= pieces[pc][1] + PIECES[pc]:
                    pc += 1
                xt_p, poff = pieces[pc]
                loc = t - poff
                pst = psts[t // 32]
                nc.tensor.matmul(out=pst[:, (t % 32) * H:(t % 32 + 1) * H],
                                 lhsT=xt_p[:, loc * 128:(loc + 1) * 128],
                                 rhs=w1_t[:], start=True, stop=True)
                if t % 32 == 31 or t == NB - 1:
                    k = t // 32
                    w = (t % 32 + 1) * H
                    sl = slice(k * 512, k * 512 + w)
                    nc.scalar.copy(out=g_sb[:, sl], in_=psts[k][:, :w])
                    nc.vector.tensor_tensor(out=g_sb[:, sl],
                                            in0=g_sb[:, sl],
                                            in1=disrep_t[:, sl],
                                            op=mybir.AluOpType.mult)
                    eng = nc.sync if k % 2 == 0 else nc.scalar
                    eng.dma_start(out=g1[:, sl], in_=g_sb[:, sl])
    nc.compile()
    return nc


def _sub_cg(cap, M, sub_cols):
    """Largest divisor of cap whose [cg, M] slab fits in sub_cols."""
    best = cap
    for dv in range(1, cap + 1):
        if cap % dv == 0 and dv * M <= sub_cols:
            best = dv
    return best


def _reduce_stream(nc, sb, segs, d, slots, lo_max, res_t, sub_cols=None,
                   epi=None, bsplit=None):
    """Per segment: per sub-chunk [DMA; halving chain], then combine the
    partials into res_t.  Emission order matches data-landing order so
    the in-order DVE queue never head-of-line blocks.  `epi(ba, bb)` is
    called right after the segment that completes block `bsplit`."""
    ring = 0
    done_b = 0
    epi_done = False
    for (cap, b0, nb, soff) in segs:
        M = nb * d
        st = sb.tile([128, lo_max], BF, tag="slotbuf")
        cg = (cap if sub_cols is None or cap * M <= sub_cols
              else _sub_cg(cap, M, sub_cols))
        G = cap // cg
        res = res_t[:, b0 * d:(b0 + nb) * d]
        for g in range(G):
            base = g * cg * M
            nc.sync.dma_start(out=st[:, base:base + cg * M],
                              in_=slots[:, soff + base:soff + base + cg * M])
            c = _halve_inplace(nc.vector, st, base, cg, M, 2)
            if G == 1:
                if c == 2:
                    nc.vector.tensor_tensor(
                        out=res, in0=st[:, :M], in1=st[:, M:2 * M],
                        op=mybir.AluOpType.add)
                else:
                    nc.vector.tensor_copy(out=res, in_=st[:, :M])
            elif c == 2:
                nc.vector.tensor_tensor(
                    out=st[:, base:base + M], in0=st[:, base:base + M],
                    in1=st[:, base + M:base + 2 * M],
                    op=mybir.AluOpType.add)
        if G > 1:
            gv = st[:, :G * cg * M].rearrange("p (g w) -> p g w",
                                              g=G, w=cg * M)
            Gc = G
            while Gc > 2:
                h = Gc // 2
                nc.vector.tensor_tensor(out=gv[:, 0:h, 0:M],
                                        in0=gv[:, 0:h, 0:M],
                                        in1=gv[:, Gc - h:Gc, 0:M],
                                        op=mybir.AluOpType.add)
                Gc -= h
            nc.vector.tensor_tensor(out=res, in0=st[:, :M],
                                    in1=st[:, cg * M:cg * M + M],
                                    op=mybir.AluOpType.add)
        done_b = b0 + nb
        if epi is not None and not epi_done and done_b >= bsplit:
            epi(0, done_b)
            epi_done = True
            bsplit_actual = done_b
    if epi is not None:
        epi(bsplit_actual if epi_done else 0, NB)


def _halve_inplace(eng, st, base, c, M, stop_at):
    """In-place halving tree on the [c, M] slab at column `base`;
    reduces down to `stop_at` slots."""
    while c > stop_at:
        h = c // 2
        eng.tensor_tensor(
            out=st[:, base:base + h * M],
            in0=st[:, base:base + h * M],
            in1=st[:, base + (c - h) * M:base + c * M],
            op=mybir.AluOpType.add)
        c -= h
    return c


def _build_l2(segs, cols, lo_max, has_b1):
    """conv1 reduce + fused epilogue.

    relu(dis*agg + b1) * w2 == relu(agg + b1/dis) * (dis*w2)  (dis > 0).
    dw2rep = dis*w2 replicated is built on the idle GPSIMD from dis1 and
    a [128, H] w2 row; the epilogue is one scalar_tensor_tensor +
    channel-tree + final dis multiply, emitted in two halves so the
    first half overlaps the tail of the slot stream.
    """
    nc = bacc.Bacc("TRN2", target_bir_lowering=False, debug=False,
                   num_devices=NC)
    slots = nc.dram_tensor("slots", [128, cols], BF,
                           kind="ExternalInput").ap()
    w2row = nc.dram_tensor("w2row", [128, H], BF, kind="ExternalInput").ap()
    dis1 = nc.dram_tensor("dis1", [128, NB], BF, kind="ExternalInput").ap()
    if has_b1:
        b1dd = nc.dram_tensor("b1dd", [128, NB * H], BF,
                              kind="ExternalInput").ap()
    g2 = nc.dram_tensor("g2", [128, NB], BF, kind="ExternalOutput").ap()
    # epilogue halves split at a segment boundary near half the blocks
    bsplit = NB // 2
    for (cap, b0, nb, off) in segs:
        if b0 <= NB // 2 <= b0 + nb:
            bsplit = b0 + nb
            break
    with tile.TileContext(nc) as tc:
        with tc.tile_pool(name="sb", bufs=min(3, len(segs))) as sb, \
             tc.tile_pool(name="cst", bufs=1) as cst:
            res_t = cst.tile([128, NB * H], BF)
            dw2rep_t = cst.tile([128, NB * H], BF)
            dis1_t = cst.tile([128, NB], BF)
            w2row_t = cst.tile([128, H], BF)
            g2_t = cst.tile([128, NB], BF)
            if has_b1:
                b1dd_t = cst.tile([128, NB * H], BF)
            nc.scalar.dma_start(out=dis1_t[:], in_=dis1[:])
            nc.scalar.dma_start(out=w2row_t[:], in_=w2row[:])
            if has_b1:
                nc.scalar.dma_start(out=b1dd_t[:], in_=b1dd[:])
            # dw2rep[p, b*H+c] = dis1[p, b] * w2[c]  (on GPSIMD, off the
            # DVE critical path)
            nc.gpsimd.tensor_tensor(
                out=dw2rep_t[:].rearrange("p (b c) -> p b c", b=NB, c=H),
                in0=dis1_t[:].unsqueeze(2).to_broadcast([128, NB, H]),
                in1=w2row_t[:].unsqueeze(1).to_broadcast([128, NB, H]),
                op=mybir.AluOpType.mult)

            def epi(ba, bb):
                if ba >= bb:
                    return
                nbh = bb - ba
                sl = slice(ba * H, bb * H)
                if has_b1:
                    nc.vector.tensor_tensor(out=res_t[:, sl],
                                            in0=res_t[:, sl],
                                            in1=b1dd_t[:, sl],
                                            op=mybir.AluOpType.add)
                nc.vector.scalar_tensor_tensor(
                    out=res_t[:, sl], in0=res_t[:, sl], scalar=0.0,
                    in1=dw2rep_t[:, sl],
                    op0=mybir.AluOpType.max, op1=mybir.AluOpType.mult)
                r3 = res_t[:, sl].rearrange("p (b c) -> p b c", b=nbh, c=H)
                w = H
                while w > 2:
                    h = w // 2
                    nc.vector.tensor_tensor(out=r3[:, :, 0:h],
                                            in0=r3[:, :, 0:h],
                                            in1=r3[:, :, w - h:w],
                                            op=mybir.AluOpType.add)
                    w -= h
                g2v = g2_t[:, ba:bb].rearrange("p (b o) -> p b o",
                                               b=nbh, o=1)
                nc.vector.tensor_tensor(out=g2v, in0=r3[:, :, 0:1],
                                        in1=r3[:, :, 1:2],
                                        op=mybir.AluOpType.add)
                nc.vector.tensor_tensor(out=g2_t[:, ba:bb],
                                        in0=g2_t[:, ba:bb],
                                        in1=dis1_t[:, ba:bb],
                                        op=mybir.AluOpType.mult)

            _reduce_stream(nc, sb, segs, H, slots, lo_max, res_t,
                           sub_cols=SUB_COLS16, epi=epi, bsplit=bsplit)
            nc.sync.dma_start(out=g2[:], in_=g2_t[:])
    nc.compile()
    return nc


def _build_l3(segs, cols, lo_max, has_b2):
    nc = bacc.Bacc("TRN2", target_bir_lowering=False, debug=False,
                   num_devices=NC)
    slots = nc.dram_tensor("slots", [128, cols], BF,
                           kind="ExternalInput").ap()
    dis1 = nc.dram_tensor("dis1", [128, NB], BF, kind="ExternalInput").ap()
    b2c = nc.dram_tensor("b2c", [128, 1], FT, kind="ExternalInput").ap()
    out = nc.dram_tensor("out", [128, NB], FT, kind="ExternalOutput").ap()
    with tile.TileContext(nc) as tc:
        with tc.tile_pool(name="sb", bufs=min(4, len(segs))) as sb, \
             tc.tile_pool(name="cst", bufs=1) as cst:
            res_t = cst.tile([128, NB], BF)
            dis1_t = cst.tile([128, NB], BF)
            b2_t = cst.tile([128, 1], FT)
            out_t = cst.tile([128, NB], FT)
            nc.scalar.dma_start(out=dis1_t[:], in_=dis1[:])
            nc.scalar.dma_start(out=b2_t[:], in_=b2c[:])
            _reduce_stream(nc, sb, segs, 1, slots, lo_max, res_t)
            nc.vector.tensor_tensor(out=out_t[:], in0=res_t[:],
                                    in1=dis1_t[:], op=mybir.AluOpType.mult)
            if has_b2:
                nc.vector.tensor_scalar(out=out_t[:], in0=out_t[:],
                                        scalar1=b2_t[:, 0:1], scalar2=None,
                                        op0=mybir.AluOpType.add)
            nc.sync.dma_start(out=out[:], in_=out_t[:])
    nc.compile()
    return nc


def _run(nc, in_maps, label):
    trace = os.environ.get("GCN_TRACE", "0") == "1"
    res = bass_utils.run_bass_kernel_spmd(nc, in_maps,
                                          core_ids=list(range(NC)),
                                          trace=trace)
    if res.exec_time_ns is not None:
        last_exec_ns[label] = res.exec_time_ns
    return res.results


# ---------------------------------------------------------------------
# host orchestration
# ---------------------------------------------------------------------
def kernel(x, edge_index, W1, b1, W2, b2):
    import ml_dtypes
    BFH = ml_dtypes.bfloat16
    x = np.asarray(x, np.float32)
    edge_index = np.asarray(edge_index, np.int32)
    W1 = np.asarray(W1, np.float32)
    b1 = np.asarray(b1, np.float32)
    W2 = np.asarray(W2, np.float32)
    b2 = np.asarray(b2, np.float32)

    # ---- host routing (integer index work) ----
    loop = np.arange(N, dtype=np.int64)
    src = np.concatenate([edge_index[0].astype(np.int64), loop])
    dst = np.concatenate([edge_index[1].astype(np.int64), loop])
    deg = np.bincount(dst, minlength=N).astype(np.int64)
    order = np.argsort(dst, kind="stable")
    src_s, dst_s = src[order], dst[order]
    core_start = np.searchsorted(dst_s, np.arange(0, N + 1, SH))

    # per-core degree-sorted row assignment + shared per-block slot caps
    pi = []           # pi[c][r] = global node id at row r (-1 = pad)
    caps_core = np.zeros((NC, NB), np.int64)
    for c in range(NC):
        d_loc = np.zeros(SHP, np.int64)
        d_loc[:SH] = deg[c * SH:(c + 1) * SH]
        ids = np.full(SHP, -1, np.int64)
        ids[:SH] = np.arange(c * SH, (c + 1) * SH)
        o = np.argsort(d_loc, kind="stable")
        pi.append(ids[o])
        caps_core[c] = np.maximum(
            CAP_R,
            ((d_loc[o].reshape(NB, 128).max(axis=1) + CAP_R - 1)
             // CAP_R) * CAP_R)
    caps = tuple(int(v) for v in caps_core.max(axis=0))
    cols16, segs16, colbase16, stride16 = _plan_segs(caps, H, SEG_MAX16)
    cols1, segs1, colbase1, stride1 = _plan_segs(caps, 1, SEG_MAX16,
                                                 merge_min_nb=32)
    lo16 = max(c * nb * H for (c, b0, nb, off) in segs16)
    lo1 = max(c * nb for (c, b0, nb, off) in segs1)

    dis_full = np.where(deg > 0, 1.0 / np.sqrt(deg.astype(np.float64)),
                        0.0).astype(np.float32)
    has_b1 = bool(np.any(b1))
    has_b2 = bool(np.any(b2))

    # ---- L1: g1 = dis * (x @ W1) on device, node-partition layout ----
    # In fp8 mode W1 is shipped scaled by 16 (to clear the e4m3 subnormal
    # range) and the 1/16 is folded into dis1 exactly.
    XDTH = ml_dtypes.float8_e4m3fn if L1FP8 else BFH
    wscale = 16.0 if L1FP8 else 1.0
    l1 = _cached.get("l1") or _cached.setdefault("l1", _build_l1())
    in_maps1 = []
    for c in range(NC):
        xs = np.zeros((SHP, F), np.float32)
        xs[:SH] = x[c * SH:(c + 1) * SH]
        dis_sh = np.zeros(SHP, np.float32)
        dis_sh[:SH] = dis_full[c * SH:(c + 1) * SH]
        # dis1[p, t] = dis[node t*128+p]
        d1nat = np.ascontiguousarray(dis_sh.reshape(NB, 128).T) / wscale
        in_maps1.append({"xT": np.ascontiguousarray(xs.T).astype(XDTH),
                         "w1": (W1 * wscale).astype(XDTH),
                         "dis1": d1nat.astype(BFH)})
    res1 = _run(l1, in_maps1, "l1")
    g_bf = np.zeros((N, H), BFH)
    for c in range(NC):
        arr = np.asarray(res1[c]["g1"]).reshape(128, NB, H)
        g_bf[c * SH:(c + 1) * SH] = arr.transpose(1, 0, 2).reshape(
            SHP, H)[:SH]

    # ---- per-core slot coordinates (host, reused for L2/L3) ----
    coords = []       # (p_e, col16_e, col1_e, srcs_e)
    dis_pi16 = []     # disrep in pi order  [128, NB*H]
    dis_pi1 = []      # [128, NB]
    for c in range(NC):
        rows = pi[c]
        r = np.arange(SHP)
        valid = rows >= 0
        dis_r = np.where(valid, dis_full[np.where(valid, rows, 0)],
                         0.0).astype(np.float32)
        d1 = dis_r.reshape(NB, 128).T            # [128, NB]
        dis_pi1.append(np.ascontiguousarray(d1).astype(BFH))
        dis_pi16.append(np.ascontiguousarray(
            np.repeat(d1, H, axis=1)).astype(BFH))
        rr = r[valid]
        nodes_r = rows[valid]
        st = core_start[c] + np.searchsorted(
            dst_s[core_start[c]:core_start[c + 1]], nodes_r)
        cnt = deg[nodes_r]
        rep_r = np.repeat(rr, cnt)
        w_e = np.arange(len(rep_r)) - np.repeat(np.cumsum(cnt) - cnt, cnt)
        srcs_e = src_s[np.repeat(st, cnt) + w_e]
        b_e = rep_r // 128
        p_e = (rep_r % 128).astype(np.int32)
        col16_e = colbase16[b_e] + w_e * stride16[b_e]
        col1_e = colbase1[b_e] + w_e * stride1[b_e]
        coords.append((p_e, col16_e, col1_e, srcs_e))

    # ---- L2: conv1 reduce + relu + W2 on device ----
    key2 = ("l2", caps, has_b1, POOL_FRAC)
    l2 = (_cached.get(key2)
          or _cached.setdefault(key2, _build_l2(segs16, cols16, lo16,
                                                has_b1)))
    w2row16 = np.tile(W2[:, 0][None, :], (128, 1)).astype(BFH)
    b1rep = np.tile(b1[None, :], (128, NB)).astype(np.float32)
    ch16 = np.arange(H, dtype=np.int64)
    in_maps2 = []
    for c in range(NC):
        p_e, col16_e, _, srcs_e = coords[c]
        sl = np.zeros((128, cols16), BFH)
        sl[p_e[:, None], col16_e[:, None] + ch16[None, :]] = g_bf[srcs_e]
        m = {"slots": sl, "w2row": w2row16, "dis1": dis_pi1[c]}
        if has_b1:
            d16 = dis_pi16[c].astype(np.float32)
            m["b1dd"] = np.where(d16 > 0, b1rep / np.maximum(d16, 1e-30),
                                 0.0).astype(BFH)
        in_maps2.append(m)
    res2 = _run(l2, in_maps2, "l2")
    g2_bf = np.zeros(N, BFH)
    for c in range(NC):
        g2c = np.asarray(res2[c]["g2"])          # [128, NB]
        rows = pi[c]
        r = np.arange(SHP)
        valid = rows >= 0
        g2_bf[rows[valid]] = g2c[(r % 128)[valid], (r // 128)[valid]]

    # ---- L3: conv2 reduce on device ----
    key3 = ("l3", caps, has_b2)
    l3 = (_cached.get(key3)
          or _cached.setdefault(key3, _build_l3(segs1, cols1, lo1,
                                                has_b2)))
    b2c = np.full((128, 1), float(b2[0]), np.float32)
    in_maps3 = []
    for c in range(NC):
        p_e, _, col1_e, srcs_e = coords[c]
        sl = np.zeros((128, cols1), BFH)
        sl[p_e, col1_e] = g2_bf[srcs_e]
        in_maps3.append({"slots": sl, "dis1": dis_pi1[c], "b2c": b2c})
    res3 = _run(l3, in_maps3, "l3")
    out = np.zeros((N, 1), np.float32)
    for c in range(NC):
        oc = np.asarray(res3[c]["out"])
        rows = pi[c]
        r = np.arange(SHP)
        valid = rows >= 0
        out[rows[valid], 0] = oc[(r % 128)[valid], (r // 128)[valid]]
    return out


# revision 27
# speedup vs baseline: 1.1092x; 1.1092x over previous
"""Trainium2 Bass kernel for 2-layer GCN (nn_GCN_39848706573686).

Node-sharded across 8 NeuronCores (12500 nodes/core + pad). Three SPMD
launches (host does integer routing between them; all FP math on device):
  L1: g1 = dis * (x @ W1), node-on-partition layout      (TensorE + ACT + DVE)
  L2: conv1 padded-ELL segment reduce via bf16 tree-adds
      + bias/relu/W2 epilogue                            (DVE)
  L3: conv2 padded-ELL segment reduce + bias             (DVE)

ELL slot arrays are slot-major per equal-cap segment: layout
[128 part, cap, nodes*d] so the segment reduce is a log2(cap) chain of
full-slab in-place tensor_tensor adds (DVE 2x bf16 perf mode; GPSIMD
takes the low-cap segments to unload the DVE). Segments stream and
reduce in a pipelined fashion.
"""
import os
import sys
import types
import numpy as np

# --- environment bootstrap (self-contained copy of bassboot logic) -----
for _p in ("/opt/trn_rl_repo", "/root/patched"):
    if _p not in sys.path and os.path.isdir(_p):
        sys.path.insert(0, _p)

from concourse import bass, bacc, mybir, tile  # noqa: E402
from concourse import bass_utils  # noqa: E402


def _install_ntff_hook():
    if "antenv.axon_hooks" not in sys.modules:
        mod = types.ModuleType("antenv.axon_hooks")
        _h = {}
        mod.set_axon_ntff_profile_hook = lambda h: _h.__setitem__("h", h)
        mod.get_axon_ntff_profile_hook = lambda: _h.get("h")
        sys.modules["antenv.axon_hooks"] = mod
        try:
            import antenv
            antenv.axon_hooks = mod
        except ImportError:
            pass
    mod = sys.modules["antenv.axon_hooks"]
    if mod.get_axon_ntff_profile_hook() is None:
        try:
            from trn_agent_boot.trn_boot import _ntff_profile_via_ctypes
            hook = _ntff_profile_via_ctypes("/opt/axon/libaxon_pjrt.so")
            if hook is not None:
                mod.set_axon_ntff_profile_hook(hook)
        except Exception:
            pass
    bass_utils.upload_artifacts = lambda tmpdir: str(tmpdir)


_install_ntff_hook()

# --- problem constants -------------------------------------------------
N, E, F, H = 100000, 3200000, 128, 16
NC = 8
SH = 12500                  # real nodes per core
SHP = 12544                 # padded rows per core (= 98 * 128)
NB = 98                     # node blocks of 128 per core
CAP_R = 8                   # cap rounding
SEG_MAX16 = 32768           # max slot columns per segment (d=16 plan)
SUB_COLS16 = 12288          # max slot columns per DMA sub-chunk
POOL_FRAC = float(os.environ.get("GCN_POOL", "0.0"))

FT = mybir.dt.float32
BF = mybir.dt.bfloat16
F8 = mybir.dt.float8e4
L1FP8 = os.environ.get("GCN_L1FP8", "0") == "1"
XDT = F8 if L1FP8 else BF

_cached = {}

# Track total device time across launches for test harness
last_exec_ns = {}


# ---------------------------------------------------------------------
# plan: equal-cap segments (optionally merged/split)
# ---------------------------------------------------------------------
def _runs(caps):
    runs = []
    b = 0
    while b < NB:
        b2 = b
        while b2 < NB and caps[b2] == caps[b]:
            b2 += 1
        runs.append((caps[b], b, b2 - b))
        b = b2
    return runs


def _plan_segs(caps, d, seg_max_cols, merge_min_nb=0):
    """Segment list [(cap, b0, nb, off)], consecutive in DRAM columns.

    layout inside a segment: [cap, nb*d] slot-major.
    colbase[b] + w*stride[b] + ch addresses edge slot w channel ch of
    block b.  merge_min_nb > 0 merges adjacent runs (raising cap) until a
    segment has at least that many blocks (d=1 coarse plan).
    """
    runs = _runs(caps)
    if merge_min_nb:
        merged = []
        cur = None
        for (cap, b0, nb) in runs:
            if cur is None:
                cur = [cap, b0, nb]
            else:
                cur[0] = max(cur[0], cap)
                cur[2] += nb
            if cur[2] >= merge_min_nb:
                merged.append(tuple(cur))
                cur = None
        if cur is not None:
            merged.append(tuple(cur))
        # enforce even nb (shift one block into the following segment)
        runs = []
        carry = 0
        out = []
        for i, (cap, b0, nb) in enumerate(merged):
            b0 -= carry
            nb += carry
            carry = 0
            if nb % 2 == 1 and i < len(merged) - 1:
                nb -= 1
                carry = 1
            out.append((cap, b0, nb))
        runs = [r for r in out if r[2] > 0]

    segs = []
    colbase = np.zeros(NB, np.int64)
    stride = np.zeros(NB, np.int64)
    off = 0
    for (cap, b0, nb) in runs:
        while nb > 0:
            take = min(nb, max(2, seg_max_cols // (d * cap)))
            if take % 2 == 1 and take < nb:
                take -= 1
            segs.append((cap, b0, take, off))
            for j in range(take):
                colbase[b0 + j] = off + j * d
                stride[b0 + j] = take * d
            off += take * d * cap
            b0 += take
            nb -= take
    return int(off), segs, colbase, stride


# ---------------------------------------------------------------------
# device builders
# ---------------------------------------------------------------------
def _build_l1():
    """g1 = disrep * (x @ W1) in [128 nodes, NB*16] layout."""
    PIECES = [49, 49] if L1FP8 else [33, 33, 32]    # blocks per piece
    nc = bacc.Bacc("TRN2", target_bir_lowering=False, debug=False,
                   num_devices=NC)
    xT = nc.dram_tensor("xT", [128, SHP], XDT, kind="ExternalInput").ap()
    w1 = nc.dram_tensor("w1", [128, H], XDT, kind="ExternalInput").ap()
    dis1 = nc.dram_tensor("dis1", [128, NB], BF, kind="ExternalInput").ap()
    g1 = nc.dram_tensor("g1", [128, NB * H], BF, kind="ExternalOutput").ap()
    with tile.TileContext(nc) as tc:
        with tc.tile_pool(name="sb", bufs=1) as sb, \
             tc.tile_pool(name="cst", bufs=1) as cst, \
             tc.tile_pool(name="ps", bufs=1, space="PSUM") as ps:
            w1_t = cst.tile([128, H], XDT)
            nc.scalar.dma_start(out=w1_t[:], in_=w1[:])
            dis1_t = cst.tile([128, NB], BF)
            nc.scalar.dma_start(out=dis1_t[:], in_=dis1[:])
            disrep_t = cst.tile([128, NB * H], BF)
            # replicate dis across the 16 channels on the idle GPSIMD
            nc.gpsimd.tensor_copy(
                out=disrep_t[:].rearrange("p (b c) -> p b c", b=NB, c=H),
                in_=dis1_t[:].unsqueeze(2).to_broadcast([128, NB, H]))
            g_sb = cst.tile([128, NB * H], BF)
            pieces = []
            off = 0
            pmax = max(PIECES)
            for pidx, nb_p in enumerate(PIECES):
                xt_p = sb.tile([128, pmax * 128], XDT, name=f"xtp{pidx}")
                eng = nc.sync if pidx % 2 == 0 else nc.scalar
                eng.dma_start(out=xt_p[:, :nb_p * 128],
                              in_=xT[:, off * 128:(off + nb_p) * 128])
                pieces.append((xt_p, off))
                off += nb_p
            psts = [ps.tile([128, 512], FT, space="PSUM", name=f"pst{i}")
                    for i in range(4)]
            pc = 0
            for t in range(NB):
                while t >= pieces[pc][1] + PIECES[pc]:
                    pc += 1
                xt_p, poff = pieces[pc]
                loc = t - poff
                pst = psts[t // 32]
                nc.tensor.matmul(out=pst[:, (t % 32) * H:(t % 32 + 1) * H],
                                 lhsT=xt_p[:, loc * 128:(loc + 1) * 128],
                                 rhs=w1_t[:], start=True, stop=True)
                if t % 32 == 31 or t == NB - 1:
                    k = t // 32
                    w = (t % 32 + 1) * H
                    sl = slice(k * 512, k * 512 + w)
                    nc.scalar.copy(out=g_sb[:, sl], in_=psts[k][:, :w])
                    nc.vector.tensor_tensor(out=g_sb[:, sl],
                                            in0=g_sb[:, sl],
                                            in1=disrep_t[:, sl],
                                            op=mybir.AluOpType.mult)
                    eng = nc.sync if k % 2 == 0 else nc.scalar
                    eng.dma_start(out=g1[:, sl], in_=g_sb[:, sl])
    nc.compile()
    return nc


def _sub_cg(cap, M, sub_cols):
    """Largest divisor of cap whose [cg, M] slab fits in sub_cols."""
    best = cap
    for dv in range(1, cap + 1):
        if cap % dv == 0 and dv * M <= sub_cols:
            best = dv
    return best


def _reduce_stream(nc, sb, segs, d, slots, lo_max, res_t, sub_cols=None,
                   epi=None, bsplit=None):
    """Per segment: per sub-chunk [DMA; halving chain], then combine the
    partials into res_t.  Emission order matches data-landing order so
    the in-order DVE queue never head-of-line blocks.  `epi(ba, bb)` is
    called right after the segment that completes block `bsplit`."""
    ring = 0
    done_b = 0
    epi_done = False
    for (cap, b0, nb, soff) in segs:
        M = nb * d
        st = sb.tile([128, lo_max], BF, tag="slotbuf")
        cg = (cap if sub_cols is None or cap * M <= sub_cols
              else _sub_cg(cap, M, sub_cols))
        G = cap // cg
        res = res_t[:, b0 * d:(b0 + nb) * d]
        for g in range(G):
            base = g * cg * M
            nc.sync.dma_start(out=st[:, base:base + cg * M],
                              in_=slots[:, soff + base:soff + base + cg * M])
            c = _halve_inplace(nc.vector, st, base, cg, M, 2)
            if G == 1:
                if c == 2:
                    nc.vector.tensor_tensor(
                        out=res, in0=st[:, :M], in1=st[:, M:2 * M],
                        op=mybir.AluOpType.add)
                else:
                    nc.vector.tensor_copy(out=res, in_=st[:, :M])
            elif c == 2:
                nc.vector.tensor_tensor(
                    out=st[:, base:base + M], in0=st[:, base:base + M],
                    in1=st[:, base + M:base + 2 * M],
                    op=mybir.AluOpType.add)
        if G > 1:
            gv = st[:, :G * cg * M].rearrange("p (g w) -> p g w",
                                              g=G, w=cg * M)
            Gc = G
            while Gc > 2:
                h = Gc // 2
                nc.vector.tensor_tensor(out=gv[:, 0:h, 0:M],
                                        in0=gv[:, 0:h, 0:M],
                                        in1=gv[:, Gc - h:Gc, 0:M],
                                        op=mybir.AluOpType.add)
                Gc -= h
            nc.vector.tensor_tensor(out=res, in0=st[:, :M],
                                    in1=st[:, cg * M:cg * M + M],
                                    op=mybir.AluOpType.add)
        done_b = b0 + nb
        if epi is not None and not epi_done and done_b >= bsplit:
            epi(0, done_b)
            epi_done = True
            bsplit_actual = done_b
    if epi is not None:
        epi(bsplit_actual if epi_done else 0, NB)


def _halve_inplace(eng, st, base, c, M, stop_at):
    """In-place halving tree on the [c, M] slab at column `base`;
    reduces down to `stop_at` slots."""
    while c > stop_at:
        h = c // 2
        eng.tensor_tensor(
            out=st[:, base:base + h * M],
            in0=st[:, base:base + h * M],
            in1=st[:, base + (c - h) * M:base + c * M],
            op=mybir.AluOpType.add)
        c -= h
    return c


def _build_l2(segs, cols, lo_max, has_b1):
    """conv1 reduce + fused epilogue.

    relu(dis*agg + b1) * w2 == relu(agg + b1/dis) * (dis*w2)  (dis > 0).
    dw2rep = dis*w2 replicated is built on the idle GPSIMD from dis1 and
    a [128, H] w2 row; the epilogue is one scalar_tensor_tensor +
    channel-tree + final dis multiply, emitted in two halves so the
    first half overlaps the tail of the slot stream.
    """
    nc = bacc.Bacc("TRN2", target_bir_lowering=False, debug=False,
                   num_devices=NC)
    slots = nc.dram_tensor("slots", [128, cols], BF,
                           kind="ExternalInput").ap()
    w2row = nc.dram_tensor("w2row", [128, H], BF, kind="ExternalInput").ap()
    dis1 = nc.dram_tensor("dis1", [128, NB], BF, kind="ExternalInput").ap()
    if has_b1:
        b1dd = nc.dram_tensor("b1dd", [128, NB * H], BF,
                              kind="ExternalInput").ap()
    g2 = nc.dram_tensor("g2", [128, NB], BF, kind="ExternalOutput").ap()
    # epilogue halves split at a segment boundary near half the blocks
    bsplit = NB // 2
    for (cap, b0, nb, off) in segs:
        if b0 <= NB // 2 <= b0 + nb:
            bsplit = b0 + nb
            break
    with tile.TileContext(nc) as tc:
        with tc.tile_pool(name="sb", bufs=min(3, len(segs))) as sb, \
             tc.tile_pool(name="cst", bufs=1) as cst:
            res_t = cst.tile([128, NB * H], BF)
            dw2rep_t = cst.tile([128, NB * H], BF)
            dis1_t = cst.tile([128, NB], BF)
            w2row_t = cst.tile([128, H], BF)
            g2_t = cst.tile([128, NB], BF)
            if has_b1:
                b1dd_t = cst.tile([128, NB * H], BF)
            nc.scalar.dma_start(out=dis1_t[:], in_=dis1[:])
            nc.scalar.dma_start(out=w2row_t[:], in_=w2row[:])
            if has_b1:
                nc.scalar.dma_start(out=b1dd_t[:], in_=b1dd[:])
            # dw2rep[p, b*H+c] = dis1[p, b] * w2[c]  (on GPSIMD, off the
            # DVE critical path)
            nc.gpsimd.tensor_tensor(
                out=dw2rep_t[:].rearrange("p (b c) -> p b c", b=NB, c=H),
                in0=dis1_t[:].unsqueeze(2).to_broadcast([128, NB, H]),
                in1=w2row_t[:].unsqueeze(1).to_broadcast([128, NB, H]),
                op=mybir.AluOpType.mult)

            def epi(ba, bb):
                if ba >= bb:
                    return
                nbh = bb - ba
                sl = slice(ba * H, bb * H)
                if has_b1:
                    nc.vector.tensor_tensor(out=res_t[:, sl],
                                            in0=res_t[:, sl],
                                            in1=b1dd_t[:, sl],
                                            op=mybir.AluOpType.add)
                nc.vector.scalar_tensor_tensor(
                    out=res_t[:, sl], in0=res_t[:, sl], scalar=0.0,
                    in1=dw2rep_t[:, sl],
                    op0=mybir.AluOpType.max, op1=mybir.AluOpType.mult)
                r3 = res_t[:, sl].rearrange("p (b c) -> p b c", b=nbh, c=H)
                w = H
                while w > 2:
                    h = w // 2
                    nc.vector.tensor_tensor(out=r3[:, :, 0:h],
                                            in0=r3[:, :, 0:h],
                                            in1=r3[:, :, w - h:w],
                                            op=mybir.AluOpType.add)
                    w -= h
                g2v = g2_t[:, ba:bb].rearrange("p (b o) -> p b o",
                                               b=nbh, o=1)
                nc.vector.tensor_tensor(out=g2v, in0=r3[:, :, 0:1],
                                        in1=r3[:, :, 1:2],
                                        op=mybir.AluOpType.add)
                nc.vector.tensor_tensor(out=g2_t[:, ba:bb],
                                        in0=g2_t[:, ba:bb],
                                        in1=dis1_t[:, ba:bb],
                                        op=mybir.AluOpType.mult)

            _reduce_stream(nc, sb, segs, H, slots, lo_max, res_t,
                           sub_cols=SUB_COLS16, epi=epi, bsplit=bsplit)
            nc.sync.dma_start(out=g2[:], in_=g2_t[:])
    nc.compile()
    return nc


def _build_l3(segs, cols, lo_max, has_b2):
    nc = bacc.Bacc("TRN2", target_bir_lowering=False, debug=False,
                   num_devices=NC)
    slots = nc.dram_tensor("slots", [128, cols], BF,
                           kind="ExternalInput").ap()
    dis1 = nc.dram_tensor("dis1", [128, NB], BF, kind="ExternalInput").ap()
    b2c = nc.dram_tensor("b2c", [128, 1], FT, kind="ExternalInput").ap()
    out = nc.dram_tensor("out", [128, NB], FT, kind="ExternalOutput").ap()
    with tile.TileContext(nc) as tc:
        with tc.tile_pool(name="sb", bufs=min(4, len(segs))) as sb, \
             tc.tile_pool(name="cst", bufs=1) as cst:
            res_t = cst.tile([128, NB], BF)
            dis1_t = cst.tile([128, NB], BF)
            b2_t = cst.tile([128, 1], FT)
            out_t = cst.tile([128, NB], FT)
            nc.scalar.dma_start(out=dis1_t[:], in_=dis1[:])
            nc.scalar.dma_start(out=b2_t[:], in_=b2c[:])
            _reduce_stream(nc, sb, segs, 1, slots, lo_max, res_t)
            nc.vector.tensor_tensor(out=out_t[:], in0=res_t[:],
                                    in1=dis1_t[:], op=mybir.AluOpType.mult)
            if has_b2:
                nc.vector.tensor_scalar(out=out_t[:], in0=out_t[:],
                                        scalar1=b2_t[:, 0:1], scalar2=None,
                                        op0=mybir.AluOpType.add)
            nc.sync.dma_start(out=out[:], in_=out_t[:])
    nc.compile()
    return nc


def _run(nc, in_maps, label):
    trace = os.environ.get("GCN_TRACE", "0") == "1"
    res = bass_utils.run_bass_kernel_spmd(nc, in_maps,
                                          core_ids=list(range(NC)),
                                          trace=trace)
    if res.exec_time_ns is not None:
        last_exec_ns[label] = res.exec_time_ns
    return res.results


# ---------------------------------------------------------------------
# host orchestration
# ---------------------------------------------------------------------
def kernel(x, edge_index, W1, b1, W2, b2):
    import ml_dtypes
    BFH = ml_dtypes.bfloat16
    x = np.asarray(x, np.float32)
    edge_index = np.asarray(edge_index, np.int32)
    W1 = np.asarray(W1, np.float32)
    b1 = np.asarray(b1, np.float32)
    W2 = np.asarray(W2, np.float32)
    b2 = np.asarray(b2, np.float32)

    # ---- host routing (integer index work) ----
    loop = np.arange(N, dtype=np.int64)
    src = np.concatenate([edge_index[0].astype(np.int64), loop])
    dst = np.concatenate([edge_index[1].astype(np.int64), loop])
    deg = np.bincount(dst, minlength=N).astype(np.int64)
    order = np.argsort(dst, kind="stable")
    src_s, dst_s = src[order], dst[order]
    core_start = np.searchsorted(dst_s, np.arange(0, N + 1, SH))

    # per-core degree-sorted row assignment + shared per-block slot caps
    pi = []           # pi[c][r] = global node id at row r (-1 = pad)
    caps_core = np.zeros((NC, NB), np.int64)
    for c in range(NC):
        d_loc = np.zeros(SHP, np.int64)
        d_loc[:SH] = deg[c * SH:(c + 1) * SH]
        ids = np.full(SHP, -1, np.int64)
        ids[:SH] = np.arange(c * SH, (c + 1) * SH)
        o = np.argsort(d_loc, kind="stable")
        pi.append(ids[o])
        caps_core[c] = np.maximum(
            CAP_R,
            ((d_loc[o].reshape(NB, 128).max(axis=1) + CAP_R - 1)
             // CAP_R) * CAP_R)
    caps = tuple(int(v) for v in caps_core.max(axis=0))
    cols16, segs16, colbase16, stride16 = _plan_segs(caps, H, SEG_MAX16)
    cols1, segs1, colbase1, stride1 = _plan_segs(caps, 1, SEG_MAX16,
                                                 merge_min_nb=32)
    lo16 = max(c * nb * H for (c, b0, nb, off) in segs16)
    lo1 = max(c * nb for (c, b0, nb, off) in segs1)

    dis_full = np.where(deg > 0, 1.0 / np.sqrt(deg.astype(np.float64)),
                        0.0).astype(np.float32)
    has_b1 = bool(np.any(b1))
    has_b2 = bool(np.any(b2))

    # ---- L1: g1 = dis * (x @ W1) on device, node-partition layout ----
    # In fp8 mode W1 is shipped scaled by 16 (to clear the e4m3 subnormal
    # range) and the 1/16 is folded into dis1 exactly.
    XDTH = ml_dtypes.float8_e4m3fn if L1FP8 else BFH
    wscale = 16.0 if L1FP8 else 1.0
    l1 = _cached.get("l1") or _cached.setdefault("l1", _build_l1())
    in_maps1 = []
    for c in range(NC):
        xs = np.zeros((SHP, F), np.float32)
        xs[:SH] = x[c * SH:(c + 1) * SH]
        dis_sh = np.zeros(SHP, np.float32)
        dis_sh[:SH] = dis_full[c * SH:(c + 1) * SH]
        # dis1[p, t] = dis[node t*128+p]
        d1nat = np.ascontiguousarray(dis_sh.reshape(NB, 128).T) / wscale
        in_maps1.append({"xT": np.ascontiguousarray(xs.T).astype(XDTH),
                         "w1": (W1 * wscale).astype(XDTH),
                         "dis1": d1nat.astype(BFH)})
    res1 = _run(l1, in_maps1, "l1")
    g_bf = np.zeros((N, H), BFH)
    for c in range(NC):
        arr = np.asarray(res1[c]["g1"]).reshape(128, NB, H)
        g_bf[c * SH:(c + 1) * SH] = arr.transpose(1, 0, 2).reshape(
            SHP, H)[:SH]

    # ---- per-core slot coordinates (host, reused for L2/L3) ----
    coords = []       # (p_e, col16_e, col1_e, srcs_e)
    dis_pi16 = []     # disrep in pi order  [128, NB*H]
    dis_pi1 = []      # [128, NB]
    for c in range(NC):
        rows = pi[c]
        r = np.arange(SHP)
        valid = rows >= 0
        dis_r = np.where(valid, dis_full[np.where(valid, rows, 0)],
                         0.0).astype(np.float32)
        d1 = dis_r.reshape(NB, 128).T            # [128, NB]
        dis_pi1.append(np.ascontiguousarray(d1).astype(BFH))
        dis_pi16.append(np.ascontiguousarray(
            np.repeat(d1, H, axis=1)).astype(BFH))
        rr = r[valid]
        nodes_r = rows[valid]
        st = core_start[c] + np.searchsorted(
            dst_s[core_start[c]:core_start[c + 1]], nodes_r)
        cnt = deg[nodes_r]
        rep_r = np.repeat(rr, cnt)
        w_e = np.arange(len(rep_r)) - np.repeat(np.cumsum(cnt) - cnt, cnt)
        srcs_e = src_s[np.repeat(st, cnt) + w_e]
        b_e = rep_r // 128
        p_e = (rep_r % 128).astype(np.int32)
        col16_e = colbase16[b_e] + w_e * stride16[b_e]
        col1_e = colbase1[b_e] + w_e * stride1[b_e]
        coords.append((p_e, col16_e, col1_e, srcs_e))

    # ---- L2: conv1 reduce + relu + W2 on device ----
    key2 = ("l2", caps, has_b1, POOL_FRAC)
    l2 = (_cached.get(key2)
          or _cached.setdefault(key2, _build_l2(segs16, cols16, lo16,
                                                has_b1)))
    w2row16 = np.tile(W2[:, 0][None, :], (128, 1)).astype(BFH)
    b1rep = np.tile(b1[None, :], (128, NB)).astype(np.float32)
    ch16 = np.arange(H, dtype=np.int64)
    in_maps2 = []
    for c in range(NC):
        p_e, col16_e, _, srcs_e = coords[c]
        sl = np.zeros((128, cols16), BFH)
        sl[p_e[:, None], col16_e[:, None] + ch16[None, :]] = g_bf[srcs_e]
        m = {"slots": sl, "w2row": w2row16, "dis1": dis_pi1[c]}
        if has_b1:
            d16 = dis_pi16[c].astype(np.float32)
            m["b1dd"] = np.where(d16 > 0, b1rep / np.maximum(d16, 1e-30),
                                 0.0).astype(BFH)
        in_maps2.append(m)
    res2 = _run(l2, in_maps2, "l2")
    g2_bf = np.zeros(N, BFH)
    for c in range(NC):
        g2c = np.asarray(res2[c]["g2"])          # [128, NB]
        rows = pi[c]
        r = np.arange(SHP)
        valid = rows >= 0
        g2_bf[rows[valid]] = g2c[(r % 128)[valid], (r // 128)[valid]]

    # ---- L3: conv2 reduce on device ----
    key3 = ("l3", caps, has_b2)
    l3 = (_cached.get(key3)
          or _cached.setdefault(key3, _build_l3(segs1, cols1, lo1,
                                                has_b2)))
    b2c = np.full((128, 1), float(b2[0]), np.float32)
    in_maps3 = []
    for c in range(NC):
        p_e, _, col1_e, srcs_e = coords[c]
        sl = np.zeros((128, cols1), BFH)
        sl[p_e, col1_e] = g2_bf[srcs_e]
        in_maps3.append({"slots": sl, "dis1": dis_pi1[c], "b2c": b2c})
    res3 = _run(l3, in_maps3, "l3")
    out = np.zeros((N, 1), np.float32)
    for c in range(NC):
        oc = np.asarray(res3[c]["out"])
        rows = pi[c]
        r = np.arange(SHP)
        valid = rows >= 0
        out[rows[valid], 0] = oc[(r % 128)[valid], (r // 128)[valid]]
    return out
